# revision 60
# baseline (speedup 1.0000x reference)
"""Causal sparse (sliding-window) attention for Trainium2, 8 NeuronCores.

Sharding: tensor-parallel over heads (16 heads -> 2 per core).  Each core
computes the qkv projection for its 2 heads (w_qkv column-parallel), windowed
causal attention, and a partial output projection (w_out row-parallel).
The host sums the 8 partial outputs (bf16 partials, fp32 accumulate).

v3 layout strategy (all-bf16 matmul inputs, fp32 psum accumulate):
  xT [D, L] bf16 streamed per 512-column chunk (prefetched two chunks ahead)
  q/k projected transposed: qT/kT [hd (2 heads packed on partitions), L]
  v projected directly in natural [keys, hd] layout (lhsT = x chunk): no
  transposes; ones columns for the softmax denominator live in v_sb
  RoPE without any rotate matmul: psum -> bf16 copy (DVE), then
    qc = raw * cos           (DVE, 2x bf16)
    qs = swap-pairs(raw)*snm (DVE, one reversed-inner-stride TT, 2x bf16)
    qrot = qc + qs           (GPSIMD, sbuf-only)
  scoresT [k, q] bf16 matmuls; exp on ACT; 0/1 bf16 masks on GPSIMD
  AV: v augmented with ones-column -> denominator lands in psum row 64
  normalization: reciprocal (DVE) + partition broadcast (GPSIMD) + mult (DVE)
  out projection: ctxT bf16 as lhsT; psum->bf16 copies alternate ACT/DVE;
  bf16 partial output DMA.  Emission order per iteration: attention(sb n-1)
  with outproj(sb n-2) interleaved as PE filler, then qkv chunk n, so the
  PE never sits behind a cross-phase dependency chain.
"""
import numpy as np

import concourse.bacc as bacc
import concourse.tile as tile
import concourse.mybir as mybir
from concourse.bass_utils import run_bass_kernel_spmd

F32 = mybir.dt.float32
BF16 = mybir.dt.bfloat16

D = 1024
L = 4096
HD = 64
N_CORES = 8
WINDOW = 512
ROPE_BASE = 10000.0
NSB = L // 512          # superblocks of 512 queries
NQB = L // 128          # 128-query blocks


def _attn_plan(sb):
    """Per-superblock key-block plan: (abs key block, lo, hi, diag_qi, far_qi).
    lo/hi bound the valid query blocks (in 0..4) for that key block; diag/far
    mark which query block needs the triangular partial mask."""
    if sb == 0:
        return [(kb, kb, 4, kb, None) for kb in range(4)]
    plan = []
    for ki in (4, 0, 1, 2, 3, 5, 6, 7):   # ki=4 first: full span, start=True
        plan.append((sb * 4 - 4 + ki, max(0, ki - 4), min(3, ki) + 1,
                     ki - 4 if ki >= 4 else None, ki if ki <= 3 else None))
    return plan


_TAGS = {}


def _tag(ret, label):
    try:
        _TAGS[ret.ins.name] = label
    except Exception:
        pass
    return ret


def _build_nc(phases=("qkv", "attn", "out"), iters=1):
    _TAGS.clear()
    nc = bacc.Bacc(None, target_bir_lowering=False)

    xT = nc.dram_tensor("xT", [D, L], BF16, kind="ExternalInput")
    wl = nc.dram_tensor("wl", [D, 384], BF16, kind="ExternalInput")
    wo = nc.dram_tensor("wo", [128, D], BF16, kind="ExternalInput")
    p2 = nc.dram_tensor("p2", [128, 128], BF16, kind="ExternalInput")
    cs = nc.dram_tensor("cs", [128, L], BF16, kind="ExternalInput")
    snm = nc.dram_tensor("snm", [128, L], BF16, kind="ExternalInput")
    md = nc.dram_tensor("md", [128, 128], BF16, kind="ExternalInput")
    mf = nc.dram_tensor("mf", [128, 128], BF16, kind="ExternalInput")
    ident = nc.dram_tensor("ident", [128, 128], BF16, kind="ExternalInput")
    po = nc.dram_tensor("po", [L, D], BF16, kind="ExternalOutput")

    xT3 = xT.rearrange("(ko ki) l -> ki ko l", ki=128)   # [128, 8, L]
    wl3 = wl.rearrange("(ko ki) m -> ki ko m", ki=128)   # [128, 8, 384]

    with tile.TileContext(nc) as tc:
        with tc.tile_pool(name="singles", bufs=1) as singles, \
             tc.tile_pool(name="work", bufs=2) as work, \
             tc.tile_pool(name="ptp", bufs=9) as ptp, \
             tc.tile_pool(name="outp", bufs=6) as outp, \
             tc.tile_pool(name="ps", bufs=2, space="PSUM") as ps:

            w_sb = singles.tile([128, 8, 384], BF16)
            p2_sb = singles.tile([128, 128], BF16)
            wo_sb = singles.tile([128, D], BF16)
            cs_sb = singles.tile([128, L], BF16)
            snm_sb = singles.tile([128, L], BF16)
            md_sb = singles.tile([128, 128], BF16)
            mf_sb = singles.tile([128, 128], BF16)
            id_sb = singles.tile([128, 128], BF16)

            qrot_sb = singles.tile([128, L], BF16)
            krot_sb = singles.tile([128, L], BF16)
            ctxT_sb = singles.tile([128, L], BF16)
            # v natural layout per 128-key block: [h0 v(64) | 1 | h1 v(64) | 1]
            v_sb = singles.tile([128, NQB, 130], BF16)

            xts = {}

            def xt_dma(n):
                xts[n] = work.tile([128, 8, 512], BF16, tag="xt", bufs=3,
                                   name=f"xt{n}")
                nc.sync.dma_start(xts[n][:], xT3[:, :, n * 512:(n + 1) * 512])

            def tab_dma(n):
                span = slice(n * 512, (n + 1) * 512)
                nc.sync.dma_start(cs_sb[:, span], cs[:, span])
                nc.sync.dma_start(snm_sb[:, span], snm[:, span])

            def tab_rest_dma():
                span = slice(512, L)
                nc.sync.dma_start(cs_sb[:, span], cs[:, span])
                nc.sync.dma_start(snm_sb[:, span], snm[:, span])

            def emit_head():
                nc.gpsimd.memset(v_sb[:, :, 64:65], 1.0)
                nc.gpsimd.memset(v_sb[:, :, 129:130], 1.0)
                # interleave w and first-x slices so the first matmul can
                # start after ~2 small transfers instead of the full load
                xts[0] = work.tile([128, 8, 512], BF16, tag="xt", bufs=3,
                                   name="xt0")
                for k8 in range(8):
                    nc.sync.dma_start(w_sb[:, k8, :], wl3[:, k8, :])
                    nc.sync.dma_start(xts[0][:, k8, :], xT3[:, k8, 0:512])
                tab_dma(0)
                nc.sync.dma_start(p2_sb[:], p2[:])
                # dummy matmuls on a zeroed tile while the first DMAs land:
                # the PE pstate ramps with continuous busy time, so real
                # matmuls start at mid/full clock instead of 0.65 GHz
                wu = work.tile([128, 512], BF16, tag="wu", bufs=1, name="wu")
                nc.gpsimd.memset(wu[:], 0.0)
                for i in range(8):
                    wup = ps.tile([128, 512], F32, tag="out", name="wup")
                    _tag(nc.tensor.matmul(wup[:], wu[:, 0:128], wu[:],
                                          start=True, stop=True),
                         f"warm {i}")

            def emit_const_dmas():
                nc.sync.dma_start(md_sb[:], md[:])
                nc.sync.dma_start(mf_sb[:], mf[:])
                nc.sync.dma_start(id_sb[:], ident[:])
                nc.sync.dma_start(wo_sb[:], wo[:])
                tab_rest_dma()

            def emit_qkv_chunk(n):
                span = slice(n * 512, (n + 1) * 512)
                xt = xts[n]
                if n + 2 < NSB:
                    xt_dma(n + 2)

                # q/k/v projections (transposed layout); psum -> bf16 copies
                # on DVE (q,k) / ACT (v).  For chunk 0 the k8 accumulation
                # steps of all three are interleaved so every arriving
                # x-slice feeds three back-to-back matmuls: the PE stays
                # busy (clock ramped) instead of idling between slices.
                psqs = [ps.tile([128, 512], F32, tag="mm", name="psq")
                        for _ in range(2)]
                vps = ps.tile([128, 512], F32, tag="out" if n == 0 else "mm",
                              name="vps")
                mms = [(psqs[0], 0), (psqs[1], 128), (vps, 256)]
                if n == 0:
                    for k8 in range(8):
                        for dst, c0 in mms:
                            _tag(nc.tensor.matmul(
                                dst[:], w_sb[:, k8, c0:c0 + 128],
                                xt[:, k8, :],
                                start=(k8 == 0), stop=(k8 == 7)),
                                f"qkvmm n{n} c{c0} k{k8}")
                else:
                    for dst, c0 in mms:
                        for k8 in range(8):
                            _tag(nc.tensor.matmul(
                                dst[:], w_sb[:, k8, c0:c0 + 128],
                                xt[:, k8, :],
                                start=(k8 == 0), stop=(k8 == 7)),
                                f"qkvmm n{n} c{c0} k{k8}")
                prj = []
                for m in range(2):
                    raw = work.tile([128, 512], BF16, tag=f"raw{m}",
                                    name="raw")
                    nc.vector.tensor_copy(raw[:], psqs[m][:])
                    prj.append(raw)
                vraw = work.tile([128, 512], BF16, tag="vraw", name="vraw")
                nc.scalar.copy(vraw[:], vps[:])

                # rotate-half matmuls (raw copies are ready by now);
                # rot_half swaps adjacent head-dims = PARTITION pairs, so it
                # must go through the PE as a permutation matmul
                rots = []
                for m in range(2):
                    psr = ps.tile([128, 512], F32, tag="out", name="psr")
                    _tag(nc.tensor.matmul(psr[:], p2_sb[:], prj[m][:],
                                          start=True, stop=True),
                         f"rotmm n{n} m{m}")
                    rots.append(psr)

                # rope combine: qc on DVE (2x bf16), qs from psum on DVE,
                # final add on GPSIMD
                for m, dst in ((0, qrot_sb), (1, krot_sb)):
                    raw = prj[m]
                    qc = work.tile([128, 512], BF16, tag=f"qc{m}", name="qc")
                    nc.vector.tensor_tensor(qc[:], raw[:], cs_sb[:, span],
                                            mybir.AluOpType.mult)
                    qs = work.tile([128, 512], BF16, tag=f"qs{m}", name="qs")
                    nc.vector.tensor_tensor(qs[:], rots[m][:],
                                            snm_sb[:, span],
                                            mybir.AluOpType.mult)
                    # first two chunks gate the first attention superblocks:
                    # run their rope-add on the fast (and still idle) DVE
                    # instead of GPSIMD to shorten the head critical chain
                    add_eng = nc.vector if n < 2 else nc.gpsimd
                    add_eng.tensor_tensor(dst[:, span], qc[:], qs[:],
                                          mybir.AluOpType.add)

                # v transposes + psum -> sbuf (DVE), ones-column gap kept:
                # one strided copy moves both head halves around the gap
                for j in range(4):
                    blk = n * 4 + j
                    tp = ps.tile([128, 128], BF16, tag="out", name="tp")
                    _tag(nc.tensor.transpose(
                        tp[:], vraw[:, j * 128:(j + 1) * 128], id_sb[:]),
                        f"vtp n{n} j{j}")
                    dst = v_sb[:, blk, 0:130].rearrange(
                        "p (two g) -> p two g", two=2)[:, :, 0:64]
                    src = tp.rearrange("p (two g) -> p two g", two=2)
                    nc.vector.tensor_copy(dst[:], src[:])

            def outproj_steps(sb):
                """Yield per-(t,nn) outproj closures for interleaving.
                The two halves of a row block share one osb tile and one
                output DMA (emitted with the second half)."""
                for ti, t in enumerate(range(sb * 4, sb * 4 + 4)):
                    osb = [None]
                    for nn in range(2):
                        def step(t=t, nn=nn, k=ti * 2 + nn, osb=osb):
                            op = ps.tile([128, 512], F32, tag="out",
                                         name="op")
                            _tag(nc.tensor.matmul(
                                op[:], ctxT_sb[:, t * 128:(t + 1) * 128],
                                wo_sb[:, nn * 512:(nn + 1) * 512],
                                start=True, stop=True), f"outmm t{t} n{nn}")
                            if osb[0] is None:
                                osb[0] = outp.tile([128, 1024], BF16,
                                                   tag="ob", name="osb")
                            half = osb[0][:, nn * 512:(nn + 1) * 512]
                            if k % 2 == 0:
                                nc.scalar.copy(half, op[:])
                            else:
                                nc.vector.tensor_copy(half, op[:])
                            if nn == 1:
                                nc.sync.dma_start(
                                    po[t * 128:(t + 1) * 128, :], osb[0][:])
                        yield step

            def emit_attention_sb(sb, fillers=(), tail_fillers=()):
                plan = _attn_plan(sb)
                n_av = len(plan)
                fillers = list(fillers)
                # consume outproj fillers from idx 2 on: gives the previous
                # superblock's norm chain slack before the first outmm
                fill_steps = max(1, n_av - 2)
                per_step = -(-len(fillers) // fill_steps) if fillers else 0
                ctxs = [ps.tile([128, 512], F32, tag="ctx", bufs=2,
                                name=f"ctx{h}") for h in range(2)]

                def emit_score(h, idx):
                    kb, lo, hi, diag_qi, far_qi = plan[idx]
                    hp = slice(h * 64, (h + 1) * 64)
                    cspan = slice(lo * 128, hi * 128)
                    qspan = slice(sb * 512 + lo * 128, sb * 512 + hi * 128)
                    scp = ps.tile([128, 512], F32, tag="sc", name="scp")
                    _tag(nc.tensor.matmul(
                        scp[:, cspan],
                        krot_sb[hp, kb * 128:(kb + 1) * 128],
                        qrot_sb[hp, qspan],
                        start=True, stop=True,
                        tile_position=(h * 64, 0)), f"scmm sb{sb} h{h} i{idx}")
                    pt = ptp.tile([128, 512], BF16, tag="pt", name="pt")
                    nc.scalar.activation(
                        pt[:, cspan], scp[:, cspan],
                        mybir.ActivationFunctionType.Exp, scale=0.125)
                    if far_qi is not None:
                        fsp = slice(far_qi * 128, (far_qi + 1) * 128)
                        nc.vector.tensor_tensor(pt[:, fsp], pt[:, fsp],
                                                mf_sb[:],
                                                mybir.AluOpType.mult)
                    if diag_qi is not None:
                        dsp = slice(diag_qi * 128, (diag_qi + 1) * 128)
                        nc.gpsimd.tensor_tensor(pt[:, dsp], pt[:, dsp],
                                                md_sb[:],
                                                mybir.AluOpType.mult)
                    return pt

                def emit_av(h, idx, pt):
                    kb, lo, hi, _, _ = plan[idx]
                    cspan = slice(lo * 128, hi * 128)
                    _tag(nc.tensor.matmul(
                        ctxs[h][0:65, cspan],
                        v_sb[:, kb, h * 65:(h + 1) * 65],
                        pt[:, cspan],
                        start=(idx == 0), stop=(idx == n_av - 1),
                        skip_group_check=True), f"avmm sb{sb} h{h} i{idx}")

                # software pipeline: AV lags the score/exp/mask chain by two
                # key blocks; outproj steps of sb-2 fill the PE in between.
                lag = min(3, n_av - 1)
                pts = {}
                for idx in range(n_av):
                    for h in range(2):
                        pts[(h, idx)] = emit_score(h, idx)
                    if idx >= 2:
                        for _ in range(per_step):
                            if fillers:
                                fillers.pop(0)()
                    if idx >= lag:
                        for h in range(2):
                            emit_av(h, idx - lag, pts.pop((h, idx - lag)))
                for idx in range(n_av - lag, n_av):
                    for h in range(2):
                        emit_av(h, idx, pts.pop((h, idx)))
                while fillers:
                    fillers.pop(0)()

                if not tail_fillers:
                    sspan = slice(sb * 512, (sb + 1) * 512)
                    for h in range(2):
                        hp = slice(h * 64, (h + 1) * 64)
                        rt = work.tile([1, 512], F32, tag="rt")
                        nc.vector.reciprocal(rt[:], ctxs[h][64:65, :])
                        rb = work.tile([64, 512], F32, tag="rb")
                        nc.gpsimd.partition_broadcast(rb[:], rt[:])
                        nc.vector.tensor_tensor(ctxT_sb[hp, sspan],
                                                ctxs[h][0:64, :],
                                                rb[:], mybir.AluOpType.mult)
                    return
                # last superblock: normalize in 128-column slices with this
                # sb's own outproj interleaved right behind each slice, so
                # the kernel tail pipelines instead of waiting for the
                # full-width norm chain
                steps = list(tail_fillers)
                rbs = {}
                for ti in range(4):
                    for h in range(2):
                        rt = work.tile([1, 128], F32, tag="rtt", bufs=4)
                        nc.vector.reciprocal(
                            rt[:], ctxs[h][64:65,
                                           ti * 128:(ti + 1) * 128])
                        rb = work.tile([64, 128], F32, tag="rbt", bufs=4)
                        nc.gpsimd.partition_broadcast(rb[:], rt[:])
                        rbs[(ti, h)] = rb
                for ti in range(4):
                    for h in range(2):
                        hp = slice(h * 64, (h + 1) * 64)
                        nc.vector.tensor_tensor(
                            ctxT_sb[hp, sb * 512 + ti * 128:
                                    sb * 512 + (ti + 1) * 128],
                            ctxs[h][0:64, ti * 128:(ti + 1) * 128],
                            rbs[(ti, h)][:], mybir.AluOpType.mult)
                    steps[2 * ti]()
                    steps[2 * ti + 1]()

            def emit_body(first):
                if first:
                    emit_head()
                else:
                    # weights/tables stay resident across loop iterations;
                    # only the streamed activations reload
                    xt_dma(0)
                if 1 < NSB:
                    xt_dma(1)
                if first:
                    emit_const_dmas()
                # attention lags qkv by 2 chunks, outproj by 1 more sb, so
                # every cross-engine chain gets a full iteration of slack
                for n in range(NSB + 3):
                    do_out = n >= 3 and "out" in phases
                    if 2 <= n <= NSB + 1 and "attn" in phases:
                        last = n == NSB + 1 and do_out
                        emit_attention_sb(
                            n - 2, outproj_steps(n - 3) if do_out else (),
                            outproj_steps(n - 2) if last else ())
                    elif do_out and n == NSB + 2 and "attn" not in phases:
                        for step in outproj_steps(n - 3):
                            step()
                    if n < NSB and "qkv" in phases:
                        emit_qkv_chunk(n)

            if iters == 1:
                emit_body(True)
            else:
                emit_head()
                emit_const_dmas()
                with tc.For_i(0, iters, 1):
                    emit_body(False)
    nc.finalize()
    return nc


def _host_constants():
    import ml_dtypes
    bf = ml_dtypes.bfloat16
    # RoPE tables, transposed + duplicated for the two packed head halves
    inv_freq = (1.0 / (ROPE_BASE ** (np.arange(0, HD, 2, dtype=np.float32)
                                     / np.float32(HD)))).astype(np.float32)
    pos = np.arange(L, dtype=np.float32)
    freqs = pos[:, None] * inv_freq[None, :]            # [L, 32]
    cos = np.repeat(np.cos(freqs), 2, axis=-1).astype(np.float32)  # [L, 64]
    sin = np.repeat(np.sin(freqs), 2, axis=-1).astype(np.float32)
    cs = np.ascontiguousarray(np.vstack([cos.T, cos.T])).astype(bf)  # [128,L]
    snT = np.ascontiguousarray(np.vstack([sin.T, sin.T])).astype(bf)

    # rotate-half as a column-space permutation: rh(q) = q @ Pc
    pc = np.zeros((HD, HD), np.float32)
    for m in range(HD // 2):
        pc[2 * m + 1, 2 * m] = -1.0
        pc[2 * m, 2 * m + 1] = 1.0
    p2 = np.zeros((128, 128), np.float32)
    p2[:64, :64] = pc
    p2[64:, 64:] = pc
    p2 = p2.astype(bf)

    k_idx = np.arange(128)[:, None]
    q_idx = np.arange(128)[None, :]
    md = (k_idx <= q_idx).astype(bf)   # diag block: valid k <= q
    mf = (k_idx > q_idx).astype(bf)    # far block: valid k > q
    ident = np.eye(128, dtype=np.float32).astype(bf)
    return cs, snT, p2, md, mf, ident


_NC_CACHE = {}


def _in_maps(x, w_qkv, w_out):
    import ml_dtypes
    bf = ml_dtypes.bfloat16
    xT = np.ascontiguousarray(x[0].T).astype(bf)       # [D, L]
    cs, snm, p2, md, mf, ident = _host_constants()
    in_maps = []
    for c in range(N_CORES):
        h0 = 2 * c
        col = slice(h0 * HD, (h0 + 2) * HD)
        wl = np.ascontiguousarray(np.concatenate(
            [w_qkv[:, 0 * D:1 * D][:, col],
             w_qkv[:, 1 * D:2 * D][:, col],
             w_qkv[:, 2 * D:3 * D][:, col]], axis=1)).astype(bf)  # [D, 384]
        wo = np.ascontiguousarray(
            w_out[h0 * HD:(h0 + 2) * HD, :]).astype(bf)  # [128, D]
        in_maps.append({"xT": xT, "wl": wl, "wo": wo, "p2": p2,
                        "cs": cs, "snm": snm, "md": md, "mf": mf,
                        "ident": ident})
    return in_maps


def kernel(x, w_qkv, w_out):
    x = np.asarray(x, np.float32)
    w_qkv = np.asarray(w_qkv, np.float32)
    w_out = np.asarray(w_out, np.float32)
    B = x.shape[0]
    assert x.shape == (B, L, D) and B == 1

    if "nc" not in _NC_CACHE:
        _NC_CACHE["nc"] = _build_nc()
    nc = _NC_CACHE["nc"]

    res = run_bass_kernel_spmd(nc, _in_maps(x, w_qkv, w_out),
                               core_ids=list(range(N_CORES)))
    out = np.zeros((L, D), np.float32)
    for r in res.results:
        out += r["po"].astype(np.float32)
    return out.astype(np.float32)[None]
